# revision 22
# baseline (speedup 1.0000x reference)
"""GCN (2-layer, hidden=64, rank-1 weights) on 8 Trainium2 NeuronCores.

Math: both GCNConv layers have rank-1 weight matrices (1->64, 64->1), so each
layer collapses to a scalar SpMV with the symmetric-normalized adjacency
A_hat = D^-1/2 (A+I) D^-1/2:

    s   = A_hat @ x                    (scalar per node)
    z   = f(s)   where f(t) = sum_k W2[k] * relu(W1[k]*t + b1[k])
    out = A_hat @ z + b2

Sharding: nodes are range-sharded by destination across the 8 cores; all
in-edges of a node live on its owner core.  Within a core, nodes are sorted
by in-degree (descending) and assigned slots COLUMN-MAJOR across the 128
SBUF partitions (slot rank k -> partition k%128, column k//128).  Round r
(the r-th in-edge of every node that has one) then occupies only
w_r = ceil(n_r/128) columns where n_r = #nodes with degree > r, so the
edge-routed tables are packed with almost no padding (~1300 columns vs
~2850 for the classic row-major ELL).  The self-loop contribution is an
extra width-98 "round" in the same table, which removes the separate
x_own/c_own inputs and their epilogue ops.

Round blocks are grouped into tiers of uniform (padded) width chosen by a
small DP; each tier is segment-summed by ONE strided vector.tensor_reduce
over a [128, u, g] access pattern (block axis innermost).

Per-edge symmetric normalization dinv[src] = rsqrt(1 + deg[src]) is computed
on the SCALAR engine as exp(-0.5*ln(1+deg)) -- Ln and Exp share one
activation-table set, and this keeps the vector engine free (the Rsqrt /
Reciprocal activations are blocked in this Bass version, and
vector.reciprocal over the edge table measured ~18us in the baseline).
Degrees are routed as uint8, values as bf16; the per-edge multiply runs in
pure bf16 (2x DVE mode), accumulation is f32.

Execution is two SPMD launches (one per GCN layer).  The host routes
per-edge source features to the owning destination core between layers
(np.take on the layer-1 activations), exactly as it routes the raw input
features for layer 1.  All arithmetic runs on the NeuronCores.
"""

import os
import numpy as np
import ml_dtypes

from concourse import bass, mybir
from concourse.bass_utils import run_bass_kernel_spmd

dt = mybir.dt
BF16 = ml_dtypes.bfloat16

NCORES = 8
N = 100000
P = 128            # SBUF partitions
CPN = 98           # node columns per partition
NPC = P * CPN      # 12544 nodes per core
SENT = NCORES * NPC  # sentinel table slot (value/deg = 0)

LAST_RESULTS = None  # list of BassKernelResults from the most recent run


def _choose_tiers(widths):
    """Group blocks (widths descending) into tiers of uniform width.
    DP minimizing ~ns: per-column cost (mult+reduce+dma) + per-tier cost."""
    B = len(widths)
    COL_NS = 2.3          # extra cost per padded column (vector+scalar+dma)
    TIER_NS = 330.0       # reduce + add instruction overhead per extra tier
    INF = float("inf")
    best = [INF] * (B + 1)
    prev = [0] * (B + 1)
    best[0] = 0.0
    for j in range(1, B + 1):
        for i in range(j):
            # tier covering blocks i..j-1, width = widths[i] (descending)
            c = best[i] + (j - i) * widths[i] * COL_NS + TIER_NS
            if c < best[j]:
                best[j] = c
                prev[j] = i
    cuts = []
    j = B
    while j > 0:
        i = prev[j]
        cuts.append((i, j))
        j = i
    cuts.reverse()
    # tier list: (col_offset, g, u); block r col start
    tiers = []
    block_col = [0] * B
    off = 0
    for (i, j) in cuts:
        u = widths[i]
        g = j - i
        for r in range(i, j):
            block_col[r] = off + (r - i) * u
        tiers.append((off, g, u))
        off += g * u
    return tiers, block_col, off


def _preprocess(x, edge_index):
    """Host routing/layout: shard by destination, degree-sort nodes
    (column-major slot order), build packed per-round source-index tables."""
    x = np.asarray(x, dtype=np.float32).reshape(-1)
    ei = np.asarray(edge_index)
    src_g = ei[0].astype(np.int64)
    dst_g = ei[1].astype(np.int64)

    cnt_g = np.bincount(dst_g, minlength=N).astype(np.int64)  # in-degree

    order_c, rank_c = [], []
    deg_sorted_c = []
    pp = np.empty(N, dtype=np.int64)  # global node -> table position
    for c in range(NCORES):
        lo, hi = c * NPC, min((c + 1) * NPC, N)
        nreal = hi - lo
        deg_local = np.zeros(NPC, dtype=np.int64)
        deg_local[:nreal] = cnt_g[lo:hi]
        order = np.argsort(-deg_local, kind="stable")
        rank = np.empty(NPC, dtype=np.int64)
        rank[order] = np.arange(NPC)
        order_c.append(order)
        rank_c.append(rank)
        deg_sorted_c.append(deg_local[order])
        pp[lo:hi] = c * NPC + rank[:nreal]

    K = int(max(int(d[0]) for d in deg_sorted_c))  # global max in-degree

    # per-round packed widths (max over cores)
    w_r = np.zeros(K, dtype=np.int64)
    for c in range(NCORES):
        ds = deg_sorted_c[c]
        for r in range(K):
            n_r = int(np.searchsorted(-ds, -(r + 1), side="right"))  # #deg>r
            w_r[r] = max(w_r[r], (n_r + P - 1) // P)

    # block 0 = self-loop block (all NPC nodes -> width CPN), then rounds
    widths = [CPN] + [int(w) for w in w_r]
    tiers, block_col, W = _choose_tiers(widths)

    # routed-table index matrices [P, W], sentinel-padded
    owner = dst_g // NPC
    idx_c = []
    for c in range(NCORES):
        lo = c * NPC
        idx_mat = np.full((P, W), SENT, dtype=np.int64)
        # self block (block 0): slot k holds node with rank k
        k_all = np.arange(NPC)
        idx_mat[k_all % P, block_col[0] + k_all // P] = c * NPC + k_all
        # edge rounds
        m = owner == c
        s_e = pp[src_g[m]]
        d_e = dst_g[m] - lo
        rj = rank_c[c][d_e]
        o = np.argsort(rj, kind="stable")
        rj_s = rj[o]
        s_s = s_e[o]
        occ = np.arange(len(rj_s)) - np.searchsorted(rj_s, rj_s)  # round idx
        cols = np.asarray(block_col, dtype=np.int64)[occ + 1] + rj_s // P
        idx_mat[rj_s % P, cols] = s_s
        idx_c.append(np.ascontiguousarray(idx_mat))

    return idx_c, order_c, cnt_g, tiers, W, K


SELF_WAITS = True  # vector self-waits: DVE drains its pipe between ops
                    # (enforced output hazard), so HW doesn't need them; set
                    # True when running under CoreSim (its race detector does
                    # not model the drain).


def _emit_tree(vector, vw, v_inc, Y, base, g, u):
    """In-place pairwise fold of g contiguous width-u blocks of Y starting at
    column `base` (pure bf16, 2x DVE mode).  Root lands at [base, base+u)."""
    n = g
    while n > 1:
        h = n // 2
        k = n - h
        vw()
        v_inc(vector.tensor_tensor(
            out=Y[:, base:base + h * u],
            in0=Y[:, base:base + h * u],
            in1=Y[:, base + k * u:base + n * u],
            op=mybir.AluOpType.add))
        n = k


def _emit_small_tiers(vector, vw, v_inc, Y, PT, tiers):
    """Fold tiers[1:] into the width-u0 root at Y[:, 0:u] (all bf16)."""
    for t in range(1, len(tiers)):
        off, g, u = tiers[t]
        if g == 1:
            vw()
            v_inc(vector.tensor_tensor(
                out=Y[:, 0:u], in0=Y[:, 0:u], in1=Y[:, off:off + u],
                op=mybir.AluOpType.add))
        elif g * u >= 128:
            _emit_tree(vector, vw, v_inc, Y, off, g, u)
            vw()
            v_inc(vector.tensor_tensor(
                out=Y[:, 0:u], in0=Y[:, 0:u], in1=Y[:, off:off + u],
                op=mybir.AluOpType.add))
        else:
            ap3 = Y[:, off:off + g * u].rearrange("p (g u) -> p u g", u=u)
            vw()
            v_inc(vector.tensor_reduce(
                out=PT[:, 0:u], in_=ap3,
                axis=mybir.AxisListType.X, op=mybir.AluOpType.add))
            vw()
            v_inc(vector.tensor_tensor(
                out=Y[:, 0:u], in0=Y[:, 0:u], in1=PT[:, 0:u],
                op=mybir.AluOpType.add))


def _tier0_groups(tiers):
    """Split tier 0's g0 blocks into pipeline groups: a small first group so
    the scalar chain starts on minimal data, then two roughly equal halves.
    Returns list of (col_lo, col_hi, nblocks)."""
    off0, g0, u0 = tiers[0]
    if g0 >= 4:
        s0 = max(2, (g0 * 3 + 2) // 5)
        sizes = [s0, g0 - s0]
    elif g0 >= 2:
        sizes = [g0 // 2, g0 - g0 // 2]
    else:
        sizes = [g0]
    groups = []
    b = 0
    for s in sizes:
        groups.append((b * u0, (b + s) * u0, s))
        b += s
    return groups


def _build_layer1(tiers, W, A, B, terms):
    """Layer 1: inputs v_ell (bf16) + c_ell (u8), both [P, W] packed tables
    including the self block.  Output [P, 2*CPN] = [w_own | dinv_own]."""
    nc = bass.Bass(num_devices=NCORES)
    nc._allow_low_precision_reason = "bf16 segment-sum within 2e-2 tolerance"

    ve_in = nc.declare_dram_parameter("v_ell", [P, W], dt.bfloat16, isOutput=False)
    ce_in = nc.declare_dram_parameter("c_ell", [P, W], dt.uint8, isOutput=False)
    out_ext = nc.declare_dram_parameter("out", [P, CPN], dt.bfloat16, isOutput=True)
    outd_ext = nc.declare_dram_parameter("outd", [P, CPN], dt.float32, isOutput=True)

    self_off = tiers[0][0]  # col offset of self block (= 0)
    off0, g0, u0 = tiers[0]
    groups = _tier0_groups(tiers)
    cC = (g0 * u0, W)
    ng = len(groups)
    umax = max([u for (_, g, u) in tiers[1:]] or [1])

    sem_cms = [nc.alloc_semaphore(f"sc{i}") for i in range(ng + 1)]
    sem_vms = [nc.alloc_semaphore(f"sx{i}") for i in range(ng + 1)]

    with (
        nc.sbuf_tensor("VE", [P, W], dt.bfloat16) as VE,
        nc.sbuf_tensor("CE", [P, W], dt.uint8) as CE,
        nc.sbuf_tensor("LN", [P, W], dt.float32) as LN,
        nc.sbuf_tensor("DE", [P, W], dt.bfloat16) as DE,
        nc.sbuf_tensor("PT", [P, umax], dt.bfloat16) as PT,
        nc.sbuf_tensor("ts", [P, CPN], dt.float32) as ts,
        nc.sbuf_tensor("to", [P, CPN], dt.float32) as to,
        nc.sbuf_tensor("tz", [P, CPN], dt.float32) as tz,
        nc.sbuf_tensor("OUT", [P, CPN], dt.bfloat16) as OUT,
        nc.sbuf_tensor("OUTD", [P, CPN], dt.float32) as OUTD,
        nc.sbuf_tensor("WRM", [P, 1], dt.float32) as WRM,
        nc.sbuf_tensor("WSCR", [P, 1], dt.float32) as WSCR,
        nc.semaphore("ss") as ss,
        nc.semaphore("sv") as sv,
        nc.semaphore("sout") as sout,
        nc.semaphore("so2") as so2,
        nc.semaphore("svd") as svd,
        nc.Block() as block,
    ):
        sv_n = [0]
        ss_n = [0]
        sca = [sem_cms[i] for i in range(ng)] + [sem_cms[ng]]
        sva = [sem_vms[i] for i in range(ng)] + [sem_vms[ng]]
        ranges = [(lo, hi) for (lo, hi, _) in groups] + [cC]

        def v_inc(inst):
            inst.then_inc(sv, 1)
            sv_n[0] += 1
            return sv_n[0]

        def s_inc(inst):
            inst.then_inc(ss, 1)
            ss_n[0] += 1
            return ss_n[0]

        @block.scalar
        def _(scalar):
            # warm up the ln/exp table set while the DMAs stream (WSCR is a
            # never-written scratch; scale=0 makes the value irrelevant)
            s_inc(scalar.activation(WRM[:, 0:1], WSCR[:, :],
                                    mybir.ActivationFunctionType.Exp,
                                    bias=0.0, scale=0.0))
            marks = []
            for i, (lo, hi) in enumerate(ranges):
                scalar.wait_ge(sca[i], 16)
                cl = s_inc(scalar.activation(LN[:, lo:hi], CE[:, lo:hi],
                                             mybir.ActivationFunctionType.Ln,
                                             bias=1.0))
                scalar.wait_ge(ss, cl)
                marks.append(s_inc(scalar.activation(
                    DE[:, lo:hi], LN[:, lo:hi],
                    mybir.ActivationFunctionType.Exp, scale=-0.5)))
            # dinv_own from the self block of LN -> OUT[:, CPN:]
            dmark = s_inc(scalar.activation(
                OUTD[:, :], LN[:, self_off:self_off + CPN],
                mybir.ActivationFunctionType.Exp, scale=-0.5))
            marks.append(dmark)
            # ship dinv early on the ACT queue (overlaps the folds); wait for
            # it so the NEFF cannot halt with the DMA in flight
            scalar.wait_ge(ss, dmark)
            scalar.dma_start(out=outd_ext[:, :],
                             in_=OUTD[:, :]).then_inc(so2, 16)
            # final w output on the ACT queue too (sync queue has higher
            # latency after streaming all the inputs)
            scalar.wait_ge(svd, 1)
            scalar.dma_start(out=out_ext[:, :],
                             in_=OUT[:, :]).then_inc(sout, 16)
            scalar.wait_ge(so2, 16)
            scalar.wait_ge(sout, 16)
            block.ss_marks = marks

        @block.vector
        def _(vector):
            dinv = OUTD[:, :]
            marks = block.ss_marks

            def vw():
                if SELF_WAITS and sv_n[0]:
                    vector.wait_ge(sv, sv_n[0])

            for i, (lo, hi, g) in enumerate(groups):
                vector.wait_ge(ss, marks[i])
                vector.wait_ge(sva[i], 16)
                vw()
                v_inc(vector.tensor_tensor(
                    out=VE[:, lo:hi], in0=VE[:, lo:hi],
                    in1=DE[:, lo:hi], op=mybir.AluOpType.mult))
            _emit_tree(vector, vw, v_inc, VE, 0, g0, u0)
            # chunk C: remaining tiers, accumulated into the bf16 root
            vector.wait_ge(ss, marks[ng])
            vector.wait_ge(sva[ng], 16)
            if cC[1] > cC[0]:
                vw()
                v_inc(vector.tensor_tensor(
                    out=VE[:, cC[0]:cC[1]], in0=VE[:, cC[0]:cC[1]],
                    in1=DE[:, cC[0]:cC[1]], op=mybir.AluOpType.mult))
            _emit_small_tiers(vector, vw, v_inc, VE, PT, tiers)
            # epilogue: s = dinv * fold ; z = f(s) ; w = dinv * z
            vector.wait_ge(ss, marks[ng + 1])
            vw()
            v_inc(vector.tensor_tensor(
                out=ts[:, :], in0=VE[:, 0:CPN], in1=dinv,
                op=mybir.AluOpType.mult))
            if terms is None:
                # z = (A-B)*relu(s) + B*s
                vw()
                v_inc(vector.tensor_scalar(
                    to[:, :], ts[:, :], 0.0, float(A - B),
                    mybir.AluOpType.max, mybir.AluOpType.mult))
                vw()
                v_inc(vector.scalar_tensor_tensor(
                    out=tz[:, :], in0=ts[:, :], scalar=float(B), in1=to[:, :],
                    op0=mybir.AluOpType.mult, op1=mybir.AluOpType.add))
            else:
                v_inc(vector.memset(tz[:, :], 0.0))
                for (w1k, b1k, w2k) in terms:
                    vw()
                    v_inc(vector.tensor_scalar(
                        to[:, :], ts[:, :], float(w1k), float(b1k),
                        mybir.AluOpType.mult, mybir.AluOpType.add))
                    vw()
                    v_inc(vector.tensor_scalar_max(to[:, :], to[:, :], 0.0))
                    vw()
                    v_inc(vector.scalar_tensor_tensor(
                        out=tz[:, :], in0=to[:, :], scalar=float(w2k),
                        in1=tz[:, :],
                        op0=mybir.AluOpType.mult, op1=mybir.AluOpType.add))
            vw()
            v_inc(vector.tensor_tensor(
                out=OUT[:, :], in0=tz[:, :], in1=dinv,
                op=mybir.AluOpType.mult))
            vector.wait_ge(sv, sv_n[0])
            vector.sem_inc(svd, 1)

        @block.sync
        def _(sync):
            # need-ordered interleave: CE_i gates the scalar chain, VE_i the
            # vector chain one act later
            for i, (lo, hi) in enumerate(ranges):
                sync.dma_start(out=CE[:, lo:hi],
                               in_=ce_in[:, lo:hi]).then_inc(sca[i], 16)
                sync.dma_start(out=VE[:, lo:hi],
                               in_=ve_in[:, lo:hi]).then_inc(sva[i], 16)
            sync.wait_ge(sout, 16)

    return nc


def _build_layer2(tiers, W, b2val):
    """Layer 2: inputs w_ell (bf16, [P, W] packed incl. self block) and
    dinv_own (f32).  out = dinv * (segment sum) + b2.
    No scalar-engine compute; the scalar engine dispatches half the DMAs so
    both HWDGE queues stream in parallel."""
    nc = bass.Bass(num_devices=NCORES)
    nc._allow_low_precision_reason = "bf16 segment-sum within 2e-2 tolerance"

    we_in = nc.declare_dram_parameter("w_ell", [P, W], dt.bfloat16, isOutput=False)
    dd_in = nc.declare_dram_parameter("dinv", [P, CPN], dt.float32, isOutput=False)
    out_ext = nc.declare_dram_parameter("out", [P, CPN], dt.float32, isOutput=True)

    off0, g0, u0 = tiers[0]
    groups = _tier0_groups(tiers)
    cC = (g0 * u0, W)
    ng = len(groups)
    umax = max([u for (_, g, u) in tiers[1:]] or [1])

    sem_w = [nc.alloc_semaphore(f"sw{i}") for i in range(ng + 1)]

    with (
        nc.sbuf_tensor("WE", [P, W], dt.bfloat16) as WE,
        nc.sbuf_tensor("DD", [P, CPN], dt.float32) as DD,
        nc.sbuf_tensor("PT", [P, umax], dt.bfloat16) as PT,
        nc.sbuf_tensor("OUT", [P, CPN], dt.float32) as OUT,
        nc.semaphore("sdd") as sdd,
        nc.semaphore("sv") as sv,
        nc.semaphore("sout") as sout,
        nc.Block() as block,
    ):
        sv_n = [0]
        ranges = [(lo, hi) for (lo, hi, _) in groups] + [cC]

        def v_inc(inst):
            inst.then_inc(sv, 1)
            sv_n[0] += 1
            return sv_n[0]

        half = (g0 // 2) * u0

        @block.scalar
        def _(scalar):
            # second HWDGE queue: second half of tier 0 + tail tiers + dinv
            scalar.dma_start(out=WE[:, half:g0 * u0],
                             in_=we_in[:, half:g0 * u0]).then_inc(sem_w[1], 16)
            scalar.dma_start(out=WE[:, cC[0]:W],
                             in_=we_in[:, cC[0]:W]).then_inc(sem_w[2], 16)
            scalar.dma_start(out=DD[:, :], in_=dd_in[:, :]).then_inc(sdd, 16)

        @block.vector
        def _(vector):
            def vw():
                if SELF_WAITS and sv_n[0]:
                    vector.wait_ge(sv, sv_n[0])

            vector.wait_ge(sem_w[0], 16)
            vector.wait_ge(sem_w[1], 16)
            _emit_tree(vector, vw, v_inc, WE, 0, g0, u0)
            vector.wait_ge(sem_w[2], 16)
            _emit_small_tiers(vector, vw, v_inc, WE, PT, tiers)
            vector.wait_ge(sdd, 16)
            vw()
            v_inc(vector.tensor_tensor(
                out=OUT[:, :], in0=WE[:, 0:CPN], in1=DD[:, :],
                op=mybir.AluOpType.mult))
            if b2val != 0.0:
                vw()
                v_inc(vector.tensor_scalar_add(OUT[:, :], OUT[:, :],
                                               float(b2val)))

        @block.sync
        def _(sync):
            sync.dma_start(out=WE[:, 0:half],
                           in_=we_in[:, 0:half]).then_inc(sem_w[0], 16)
            sync.wait_ge(sv, sv_n[0])
            sync.dma_start(out=out_ext[:, :], in_=OUT[:, :]).then_inc(sout, 16)
            sync.wait_ge(sout, 16)

    return nc


def kernel(x, edge_index, W1, b1, W2, b2):
    global LAST_RESULTS
    idx_c, order_c, cnt_g, tiers, W, K = _preprocess(x, edge_index)

    w1 = np.asarray(W1, dtype=np.float64).reshape(-1)
    w2 = np.asarray(W2, dtype=np.float64).reshape(-1)
    b1v = np.asarray(b1, dtype=np.float64).reshape(-1)
    b2v = float(np.asarray(b2, dtype=np.float64).reshape(-1)[0])
    if np.all(b1v == 0.0):
        A = float(np.sum(w2 * w1 * (w1 > 0)))
        B = float(np.sum(w2 * w1 * (w1 < 0)))
        terms = None
    else:
        A = B = 0.0
        terms = [(float(w1[k]), float(b1v[k]), float(w2[k]))
                 for k in range(len(w1))]

    # routed tables (slot-rank order per core) + sentinel 0
    xf = np.asarray(x, dtype=np.float32).reshape(-1)
    x_tab = np.zeros(SENT + 1, dtype=np.float32)
    c_tab = np.zeros(SENT + 1, dtype=np.int64)
    for c in range(NCORES):
        lo, hi = c * NPC, min((c + 1) * NPC, N)
        xv = np.zeros(NPC, dtype=np.float32)
        xv[:hi - lo] = xf[lo:hi]
        dv = np.zeros(NPC, dtype=np.int64)
        dv[:hi - lo] = cnt_g[lo:hi]
        x_tab[c * NPC:(c + 1) * NPC] = xv[order_c[c]]
        c_tab[c * NPC:(c + 1) * NPC] = dv[order_c[c]]
    x_tab16 = x_tab.astype(BF16)
    c_tab8 = c_tab.astype(np.uint8)

    trace = bool(os.environ.get("BASS_TRACE"))

    # ---- layer 1 ----
    nc1 = _build_layer1(tiers, W, A, B, terms)
    maps1 = [{
        "v_ell": np.ascontiguousarray(x_tab16[idx_c[c]]),
        "c_ell": np.ascontiguousarray(c_tab8[idx_c[c]]),
    } for c in range(NCORES)]
    res1 = run_bass_kernel_spmd(nc1, maps1, list(range(NCORES)), trace=trace)

    # host routes layer-1 message values to edge slots (halo exchange)
    w_tab16 = np.zeros(SENT + 1, dtype=BF16)
    dd_c = []
    for c in range(NCORES):
        o = np.asarray(res1.results[c]["out"])   # bf16 w
        w_tab16[c * NPC:(c + 1) * NPC] = o.T.ravel()
        dd_c.append(np.ascontiguousarray(np.asarray(res1.results[c]["outd"])))

    # ---- layer 2 ----
    nc2 = _build_layer2(tiers, W, b2v)
    maps2 = [{
        "w_ell": np.ascontiguousarray(w_tab16[idx_c[c]]),
        "dinv": dd_c[c],
    } for c in range(NCORES)]
    res2 = run_bass_kernel_spmd(nc2, maps2, list(range(NCORES)), trace=trace)

    LAST_RESULTS = [res1, res2]

    out = np.empty((N, 1), dtype=np.float32)
    for c in range(NCORES):
        lo, hi = c * NPC, min((c + 1) * NPC, N)
        o_ranked = np.asarray(res2.results[c]["out"]).T.ravel()  # value by rank
        node_of_rank = order_c[c]          # rank -> local node id
        vals = np.empty(NPC, dtype=np.float32)
        vals[node_of_rank] = o_ranked      # local node id -> value
        out[lo:hi, 0] = vals[:hi - lo]
    return out


# revision 23
# speedup vs baseline: 1.0081x; 1.0081x over previous
"""GCN (2-layer, hidden=64, rank-1 weights) on 8 Trainium2 NeuronCores.

Math: both GCNConv layers have rank-1 weight matrices (1->64, 64->1), so each
layer collapses to a scalar SpMV with the symmetric-normalized adjacency
A_hat = D^-1/2 (A+I) D^-1/2:

    s   = A_hat @ x                    (scalar per node)
    z   = f(s)   where f(t) = sum_k W2[k] * relu(W1[k]*t + b1[k])
    out = A_hat @ z + b2

Sharding: nodes are range-sharded by destination across the 8 cores; all
in-edges of a node live on its owner core.  Within a core, nodes are sorted
by in-degree (descending) and assigned slots COLUMN-MAJOR across the 128
SBUF partitions (slot rank k -> partition k%128, column k//128).  Round r
(the r-th in-edge of every node that has one) then occupies only
w_r = ceil(n_r/128) columns where n_r = #nodes with degree > r, so the
edge-routed tables are packed with almost no padding (~1300 columns vs
~2850 for the classic row-major ELL).  The self-loop contribution is an
extra width-98 "round" in the same table, which removes the separate
x_own/c_own inputs and their epilogue ops.

Round blocks are grouped into tiers of uniform (padded) width chosen by a
small DP; each tier is segment-summed by ONE strided vector.tensor_reduce
over a [128, u, g] access pattern (block axis innermost).

Per-edge symmetric normalization dinv[src] = rsqrt(1 + deg[src]) is computed
on the SCALAR engine as exp(-0.5*ln(1+deg)) -- Ln and Exp share one
activation-table set, and this keeps the vector engine free (the Rsqrt /
Reciprocal activations are blocked in this Bass version, and
vector.reciprocal over the edge table measured ~18us in the baseline).
Degrees are routed as uint8, values as bf16; the per-edge multiply runs in
pure bf16 (2x DVE mode), accumulation is f32.

Execution is two SPMD launches (one per GCN layer).  The host routes
per-edge source features to the owning destination core between layers
(np.take on the layer-1 activations), exactly as it routes the raw input
features for layer 1.  All arithmetic runs on the NeuronCores.
"""

import os
import numpy as np
import ml_dtypes

from concourse import bass, mybir
from concourse.bass_utils import run_bass_kernel_spmd

dt = mybir.dt
BF16 = ml_dtypes.bfloat16

NCORES = 8
N = 100000
P = 128            # SBUF partitions
CPN = 98           # node columns per partition
NPC = P * CPN      # 12544 nodes per core
SENT = NCORES * NPC  # sentinel table slot (value/deg = 0)

LAST_RESULTS = None  # list of BassKernelResults from the most recent run


def _choose_tiers(widths):
    """Group blocks (widths descending) into tiers of uniform width.
    DP minimizing ~ns: per-column cost (mult+reduce+dma) + per-tier cost."""
    B = len(widths)
    COL_NS = 2.3          # extra cost per padded column (vector+scalar+dma)
    TIER_NS = 330.0       # reduce + add instruction overhead per extra tier
    INF = float("inf")
    best = [INF] * (B + 1)
    prev = [0] * (B + 1)
    best[0] = 0.0
    for j in range(1, B + 1):
        for i in range(j):
            # tier covering blocks i..j-1, width = widths[i] (descending)
            c = best[i] + (j - i) * widths[i] * COL_NS + TIER_NS
            if c < best[j]:
                best[j] = c
                prev[j] = i
    cuts = []
    j = B
    while j > 0:
        i = prev[j]
        cuts.append((i, j))
        j = i
    cuts.reverse()
    # tier list: (col_offset, g, u); block r col start
    tiers = []
    block_col = [0] * B
    off = 0
    for (i, j) in cuts:
        u = widths[i]
        g = j - i
        for r in range(i, j):
            block_col[r] = off + (r - i) * u
        tiers.append((off, g, u))
        off += g * u
    return tiers, block_col, off


def _preprocess(x, edge_index):
    """Host routing/layout: shard by destination, degree-sort nodes
    (column-major slot order), build packed per-round source-index tables."""
    x = np.asarray(x, dtype=np.float32).reshape(-1)
    ei = np.asarray(edge_index)
    src_g = ei[0].astype(np.int64)
    dst_g = ei[1].astype(np.int64)

    cnt_g = np.bincount(dst_g, minlength=N).astype(np.int64)  # in-degree

    order_c, rank_c = [], []
    deg_sorted_c = []
    pp = np.empty(N, dtype=np.int64)  # global node -> table position
    for c in range(NCORES):
        lo, hi = c * NPC, min((c + 1) * NPC, N)
        nreal = hi - lo
        deg_local = np.zeros(NPC, dtype=np.int64)
        deg_local[:nreal] = cnt_g[lo:hi]
        order = np.argsort(-deg_local, kind="stable")
        rank = np.empty(NPC, dtype=np.int64)
        rank[order] = np.arange(NPC)
        order_c.append(order)
        rank_c.append(rank)
        deg_sorted_c.append(deg_local[order])
        pp[lo:hi] = c * NPC + rank[:nreal]

    K = int(max(int(d[0]) for d in deg_sorted_c))  # global max in-degree

    # per-round packed widths (max over cores)
    w_r = np.zeros(K, dtype=np.int64)
    for c in range(NCORES):
        ds = deg_sorted_c[c]
        for r in range(K):
            n_r = int(np.searchsorted(-ds, -(r + 1), side="right"))  # #deg>r
            w_r[r] = max(w_r[r], (n_r + P - 1) // P)

    # block 0 = self-loop block (all NPC nodes -> width CPN), then rounds
    widths = [CPN] + [int(w) for w in w_r]
    tiers, block_col, W = _choose_tiers(widths)

    # routed-table index matrices [P, W], sentinel-padded
    owner = dst_g // NPC
    idx_c = []
    for c in range(NCORES):
        lo = c * NPC
        idx_mat = np.full((P, W), SENT, dtype=np.int64)
        # self block (block 0): slot k holds node with rank k
        k_all = np.arange(NPC)
        idx_mat[k_all % P, block_col[0] + k_all // P] = c * NPC + k_all
        # edge rounds
        m = owner == c
        s_e = pp[src_g[m]]
        d_e = dst_g[m] - lo
        rj = rank_c[c][d_e]
        o = np.argsort(rj, kind="stable")
        rj_s = rj[o]
        s_s = s_e[o]
        occ = np.arange(len(rj_s)) - np.searchsorted(rj_s, rj_s)  # round idx
        cols = np.asarray(block_col, dtype=np.int64)[occ + 1] + rj_s // P
        idx_mat[rj_s % P, cols] = s_s
        idx_c.append(np.ascontiguousarray(idx_mat))

    return idx_c, order_c, cnt_g, tiers, W, K


SELF_WAITS = True  # vector self-waits: DVE drains its pipe between ops
                    # (enforced output hazard), so HW doesn't need them; set
                    # True when running under CoreSim (its race detector does
                    # not model the drain).


def _emit_tree(vector, vw, v_inc, Y, base, g, u):
    """In-place pairwise fold of g contiguous width-u blocks of Y starting at
    column `base` (pure bf16, 2x DVE mode).  Root lands at [base, base+u)."""
    n = g
    while n > 1:
        h = n // 2
        k = n - h
        vw()
        v_inc(vector.tensor_tensor(
            out=Y[:, base:base + h * u],
            in0=Y[:, base:base + h * u],
            in1=Y[:, base + k * u:base + n * u],
            op=mybir.AluOpType.add))
        n = k


def _emit_small_tiers(vector, vw, v_inc, Y, PT, tiers):
    """Fold tiers[1:] into the width-u0 root at Y[:, 0:u] (all bf16)."""
    for t in range(1, len(tiers)):
        off, g, u = tiers[t]
        if g == 1:
            vw()
            v_inc(vector.tensor_tensor(
                out=Y[:, 0:u], in0=Y[:, 0:u], in1=Y[:, off:off + u],
                op=mybir.AluOpType.add))
        elif g * u >= 128:
            _emit_tree(vector, vw, v_inc, Y, off, g, u)
            vw()
            v_inc(vector.tensor_tensor(
                out=Y[:, 0:u], in0=Y[:, 0:u], in1=Y[:, off:off + u],
                op=mybir.AluOpType.add))
        else:
            ap3 = Y[:, off:off + g * u].rearrange("p (g u) -> p u g", u=u)
            vw()
            v_inc(vector.tensor_reduce(
                out=PT[:, 0:u], in_=ap3,
                axis=mybir.AxisListType.X, op=mybir.AluOpType.add))
            vw()
            v_inc(vector.tensor_tensor(
                out=Y[:, 0:u], in0=Y[:, 0:u], in1=PT[:, 0:u],
                op=mybir.AluOpType.add))


def _tier0_groups(tiers):
    """Split tier 0's g0 blocks into pipeline groups: a small first group so
    the scalar chain starts on minimal data, then two roughly equal halves.
    Returns list of (col_lo, col_hi, nblocks)."""
    off0, g0, u0 = tiers[0]
    if g0 >= 4:
        s0 = max(2, (g0 * 3 + 2) // 5)
        sizes = [s0, g0 - s0]
    elif g0 >= 2:
        sizes = [g0 // 2, g0 - g0 // 2]
    else:
        sizes = [g0]
    groups = []
    b = 0
    for s in sizes:
        groups.append((b * u0, (b + s) * u0, s))
        b += s
    return groups


def _build_layer1(tiers, W, A, B, terms):
    """Layer 1: inputs v_ell (bf16) + c_ell (u8), both [P, W] packed tables
    including the self block.  Output [P, 2*CPN] = [w_own | dinv_own]."""
    nc = bass.Bass(num_devices=NCORES)
    nc._allow_low_precision_reason = "bf16 segment-sum within 2e-2 tolerance"

    ve_in = nc.declare_dram_parameter("v_ell", [P, W], dt.bfloat16, isOutput=False)
    ce_in = nc.declare_dram_parameter("c_ell", [P, W], dt.uint8, isOutput=False)
    out_ext = nc.declare_dram_parameter("out", [P, CPN], dt.bfloat16, isOutput=True)
    outd_ext = nc.declare_dram_parameter("outd", [P, CPN], dt.float32, isOutput=True)

    self_off = tiers[0][0]  # col offset of self block (= 0)
    off0, g0, u0 = tiers[0]
    groups = _tier0_groups(tiers)
    cC = (g0 * u0, W)
    ng = len(groups)
    umax = max([u for (_, g, u) in tiers[1:]] or [1])

    sem_cms = [nc.alloc_semaphore(f"sc{i}") for i in range(ng + 1)]
    sem_vms = [nc.alloc_semaphore(f"sx{i}") for i in range(ng + 1)]

    with (
        nc.sbuf_tensor("VE", [P, W], dt.bfloat16) as VE,
        nc.sbuf_tensor("CE", [P, W], dt.uint8) as CE,
        nc.sbuf_tensor("LN", [P, W], dt.float32) as LN,
        nc.sbuf_tensor("DE", [P, W], dt.bfloat16) as DE,
        nc.sbuf_tensor("PT", [P, umax], dt.bfloat16) as PT,
        nc.sbuf_tensor("ts", [P, CPN], dt.float32) as ts,
        nc.sbuf_tensor("to", [P, CPN], dt.float32) as to,
        nc.sbuf_tensor("tz", [P, CPN], dt.float32) as tz,
        nc.sbuf_tensor("OUT", [P, CPN], dt.bfloat16) as OUT,
        nc.sbuf_tensor("OUTD", [P, CPN], dt.float32) as OUTD,
        nc.sbuf_tensor("WRM", [P, 1], dt.float32) as WRM,
        nc.sbuf_tensor("WSCR", [P, 1], dt.float32) as WSCR,
        nc.semaphore("ss") as ss,
        nc.semaphore("sv") as sv,
        nc.semaphore("sout") as sout,
        nc.semaphore("so2") as so2,
        nc.Block() as block,
    ):
        sv_n = [0]
        ss_n = [0]
        sca = [sem_cms[i] for i in range(ng)] + [sem_cms[ng]]
        sva = [sem_vms[i] for i in range(ng)] + [sem_vms[ng]]
        ranges = [(lo, hi) for (lo, hi, _) in groups] + [cC]

        def v_inc(inst):
            inst.then_inc(sv, 1)
            sv_n[0] += 1
            return sv_n[0]

        def s_inc(inst):
            inst.then_inc(ss, 1)
            ss_n[0] += 1
            return ss_n[0]

        @block.scalar
        def _(scalar):
            # warm up the ln/exp table set while the DMAs stream (WSCR is a
            # never-written scratch; scale=0 makes the value irrelevant)
            s_inc(scalar.activation(WRM[:, 0:1], WSCR[:, :],
                                    mybir.ActivationFunctionType.Exp,
                                    bias=0.0, scale=0.0))
            marks = []
            for i, (lo, hi) in enumerate(ranges):
                scalar.wait_ge(sca[i], 16)
                cl = s_inc(scalar.activation(LN[:, lo:hi], CE[:, lo:hi],
                                             mybir.ActivationFunctionType.Ln,
                                             bias=1.0))
                scalar.wait_ge(ss, cl)
                marks.append(s_inc(scalar.activation(
                    DE[:, lo:hi], LN[:, lo:hi],
                    mybir.ActivationFunctionType.Exp, scale=-0.5)))
            # dinv_own from the self block of LN -> OUT[:, CPN:]
            dmark = s_inc(scalar.activation(
                OUTD[:, :], LN[:, self_off:self_off + CPN],
                mybir.ActivationFunctionType.Exp, scale=-0.5))
            marks.append(dmark)
            # ship dinv early on the ACT queue (overlaps the folds); wait for
            # it so the NEFF cannot halt with the DMA in flight
            scalar.wait_ge(ss, dmark)
            scalar.dma_start(out=outd_ext[:, :],
                             in_=OUTD[:, :]).then_inc(so2, 16)
            scalar.wait_ge(so2, 16)
            block.ss_marks = marks

        @block.vector
        def _(vector):
            dinv = OUTD[:, :]
            marks = block.ss_marks

            def vw():
                if SELF_WAITS and sv_n[0]:
                    vector.wait_ge(sv, sv_n[0])

            for i, (lo, hi, g) in enumerate(groups):
                vector.wait_ge(ss, marks[i])
                vector.wait_ge(sva[i], 16)
                vw()
                v_inc(vector.tensor_tensor(
                    out=VE[:, lo:hi], in0=VE[:, lo:hi],
                    in1=DE[:, lo:hi], op=mybir.AluOpType.mult))
            _emit_tree(vector, vw, v_inc, VE, 0, g0, u0)
            # chunk C: remaining tiers, accumulated into the bf16 root
            vector.wait_ge(ss, marks[ng])
            vector.wait_ge(sva[ng], 16)
            if cC[1] > cC[0]:
                vw()
                v_inc(vector.tensor_tensor(
                    out=VE[:, cC[0]:cC[1]], in0=VE[:, cC[0]:cC[1]],
                    in1=DE[:, cC[0]:cC[1]], op=mybir.AluOpType.mult))
            _emit_small_tiers(vector, vw, v_inc, VE, PT, tiers)
            # epilogue: s = dinv * fold ; z = f(s) ; w = dinv * z
            vector.wait_ge(ss, marks[ng + 1])
            vw()
            v_inc(vector.tensor_tensor(
                out=ts[:, :], in0=VE[:, 0:CPN], in1=dinv,
                op=mybir.AluOpType.mult))
            if terms is None:
                # z = (A-B)*relu(s) + B*s
                vw()
                v_inc(vector.tensor_scalar(
                    to[:, :], ts[:, :], 0.0, float(A - B),
                    mybir.AluOpType.max, mybir.AluOpType.mult))
                vw()
                v_inc(vector.scalar_tensor_tensor(
                    out=tz[:, :], in0=ts[:, :], scalar=float(B), in1=to[:, :],
                    op0=mybir.AluOpType.mult, op1=mybir.AluOpType.add))
            else:
                v_inc(vector.memset(tz[:, :], 0.0))
                for (w1k, b1k, w2k) in terms:
                    vw()
                    v_inc(vector.tensor_scalar(
                        to[:, :], ts[:, :], float(w1k), float(b1k),
                        mybir.AluOpType.mult, mybir.AluOpType.add))
                    vw()
                    v_inc(vector.tensor_scalar_max(to[:, :], to[:, :], 0.0))
                    vw()
                    v_inc(vector.scalar_tensor_tensor(
                        out=tz[:, :], in0=to[:, :], scalar=float(w2k),
                        in1=tz[:, :],
                        op0=mybir.AluOpType.mult, op1=mybir.AluOpType.add))
            vw()
            v_inc(vector.tensor_tensor(
                out=OUT[:, :], in0=tz[:, :], in1=dinv,
                op=mybir.AluOpType.mult))

        @block.sync
        def _(sync):
            # need-ordered interleave: CE_i gates the scalar chain, VE_i the
            # vector chain one act later
            for i, (lo, hi) in enumerate(ranges):
                sync.dma_start(out=CE[:, lo:hi],
                               in_=ce_in[:, lo:hi]).then_inc(sca[i], 16)
                sync.dma_start(out=VE[:, lo:hi],
                               in_=ve_in[:, lo:hi]).then_inc(sva[i], 16)
            sync.wait_ge(ss, ss_n[0])
            sync.wait_ge(sv, sv_n[0])
            sync.dma_start(out=out_ext[:, :],
                           in_=OUT[:, :]).then_inc(sout, 16)
            # the NEFF must not halt before the output DMA lands -- the host
            # result fetch races an in-flight DMA otherwise
            sync.wait_ge(sout, 16)

    return nc


def _build_layer2(tiers, W, b2val):
    """Layer 2: inputs w_ell (bf16, [P, W] packed incl. self block) and
    dinv_own (f32).  out = dinv * (segment sum) + b2.
    No scalar-engine compute; the scalar engine dispatches half the DMAs so
    both HWDGE queues stream in parallel."""
    nc = bass.Bass(num_devices=NCORES)
    nc._allow_low_precision_reason = "bf16 segment-sum within 2e-2 tolerance"

    we_in = nc.declare_dram_parameter("w_ell", [P, W], dt.bfloat16, isOutput=False)
    dd_in = nc.declare_dram_parameter("dinv", [P, CPN], dt.float32, isOutput=False)
    out_ext = nc.declare_dram_parameter("out", [P, CPN], dt.float32, isOutput=True)

    off0, g0, u0 = tiers[0]
    groups = _tier0_groups(tiers)
    cC = (g0 * u0, W)
    ng = len(groups)
    umax = max([u for (_, g, u) in tiers[1:]] or [1])

    sem_w = [nc.alloc_semaphore(f"sw{i}") for i in range(ng + 1)]

    with (
        nc.sbuf_tensor("WE", [P, W], dt.bfloat16) as WE,
        nc.sbuf_tensor("DD", [P, CPN], dt.float32) as DD,
        nc.sbuf_tensor("PT", [P, umax], dt.bfloat16) as PT,
        nc.sbuf_tensor("OUT", [P, CPN], dt.float32) as OUT,
        nc.semaphore("sdd") as sdd,
        nc.semaphore("sv") as sv,
        nc.semaphore("sout") as sout,
        nc.Block() as block,
    ):
        sv_n = [0]
        ranges = [(lo, hi) for (lo, hi, _) in groups] + [cC]

        def v_inc(inst):
            inst.then_inc(sv, 1)
            sv_n[0] += 1
            return sv_n[0]

        half = (g0 // 2) * u0

        @block.scalar
        def _(scalar):
            # second HWDGE queue: second half of tier 0 + tail tiers + dinv
            scalar.dma_start(out=WE[:, half:g0 * u0],
                             in_=we_in[:, half:g0 * u0]).then_inc(sem_w[1], 16)
            scalar.dma_start(out=WE[:, cC[0]:W],
                             in_=we_in[:, cC[0]:W]).then_inc(sem_w[2], 16)
            scalar.dma_start(out=DD[:, :], in_=dd_in[:, :]).then_inc(sdd, 16)

        @block.vector
        def _(vector):
            def vw():
                if SELF_WAITS and sv_n[0]:
                    vector.wait_ge(sv, sv_n[0])

            vector.wait_ge(sem_w[0], 16)
            vector.wait_ge(sem_w[1], 16)
            _emit_tree(vector, vw, v_inc, WE, 0, g0, u0)
            vector.wait_ge(sem_w[2], 16)
            _emit_small_tiers(vector, vw, v_inc, WE, PT, tiers)
            vector.wait_ge(sdd, 16)
            vw()
            v_inc(vector.tensor_tensor(
                out=OUT[:, :], in0=WE[:, 0:CPN], in1=DD[:, :],
                op=mybir.AluOpType.mult))
            if b2val != 0.0:
                vw()
                v_inc(vector.tensor_scalar_add(OUT[:, :], OUT[:, :],
                                               float(b2val)))

        @block.sync
        def _(sync):
            sync.dma_start(out=WE[:, 0:half],
                           in_=we_in[:, 0:half]).then_inc(sem_w[0], 16)
            sync.wait_ge(sv, sv_n[0])
            sync.dma_start(out=out_ext[:, :], in_=OUT[:, :]).then_inc(sout, 16)
            sync.wait_ge(sout, 16)

    return nc


def kernel(x, edge_index, W1, b1, W2, b2):
    global LAST_RESULTS
    idx_c, order_c, cnt_g, tiers, W, K = _preprocess(x, edge_index)

    w1 = np.asarray(W1, dtype=np.float64).reshape(-1)
    w2 = np.asarray(W2, dtype=np.float64).reshape(-1)
    b1v = np.asarray(b1, dtype=np.float64).reshape(-1)
    b2v = float(np.asarray(b2, dtype=np.float64).reshape(-1)[0])
    if np.all(b1v == 0.0):
        A = float(np.sum(w2 * w1 * (w1 > 0)))
        B = float(np.sum(w2 * w1 * (w1 < 0)))
        terms = None
    else:
        A = B = 0.0
        terms = [(float(w1[k]), float(b1v[k]), float(w2[k]))
                 for k in range(len(w1))]

    # routed tables (slot-rank order per core) + sentinel 0
    xf = np.asarray(x, dtype=np.float32).reshape(-1)
    x_tab = np.zeros(SENT + 1, dtype=np.float32)
    c_tab = np.zeros(SENT + 1, dtype=np.int64)
    for c in range(NCORES):
        lo, hi = c * NPC, min((c + 1) * NPC, N)
        xv = np.zeros(NPC, dtype=np.float32)
        xv[:hi - lo] = xf[lo:hi]
        dv = np.zeros(NPC, dtype=np.int64)
        dv[:hi - lo] = cnt_g[lo:hi]
        x_tab[c * NPC:(c + 1) * NPC] = xv[order_c[c]]
        c_tab[c * NPC:(c + 1) * NPC] = dv[order_c[c]]
    x_tab16 = x_tab.astype(BF16)
    c_tab8 = c_tab.astype(np.uint8)

    trace = bool(os.environ.get("BASS_TRACE"))

    # ---- layer 1 ----
    nc1 = _build_layer1(tiers, W, A, B, terms)
    maps1 = [{
        "v_ell": np.ascontiguousarray(x_tab16[idx_c[c]]),
        "c_ell": np.ascontiguousarray(c_tab8[idx_c[c]]),
    } for c in range(NCORES)]
    res1 = run_bass_kernel_spmd(nc1, maps1, list(range(NCORES)), trace=trace)

    # host routes layer-1 message values to edge slots (halo exchange)
    w_tab16 = np.zeros(SENT + 1, dtype=BF16)
    dd_c = []
    for c in range(NCORES):
        o = np.asarray(res1.results[c]["out"])   # bf16 w
        w_tab16[c * NPC:(c + 1) * NPC] = o.T.ravel()
        dd_c.append(np.ascontiguousarray(np.asarray(res1.results[c]["outd"])))

    # ---- layer 2 ----
    nc2 = _build_layer2(tiers, W, b2v)
    maps2 = [{
        "w_ell": np.ascontiguousarray(w_tab16[idx_c[c]]),
        "dinv": dd_c[c],
    } for c in range(NCORES)]
    res2 = run_bass_kernel_spmd(nc2, maps2, list(range(NCORES)), trace=trace)

    LAST_RESULTS = [res1, res2]

    out = np.empty((N, 1), dtype=np.float32)
    for c in range(NCORES):
        lo, hi = c * NPC, min((c + 1) * NPC, N)
        o_ranked = np.asarray(res2.results[c]["out"]).T.ravel()  # value by rank
        node_of_rank = order_c[c]          # rank -> local node id
        vals = np.empty(NPC, dtype=np.float32)
        vals[node_of_rank] = o_ranked      # local node id -> value
        out[lo:hi, 0] = vals[:hi - lo]
    return out


# revision 24
# speedup vs baseline: 1.0346x; 1.0263x over previous
"""GCN (2-layer, hidden=64, rank-1 weights) on 8 Trainium2 NeuronCores.

Math: both GCNConv layers have rank-1 weight matrices (1->64, 64->1), so each
layer collapses to a scalar SpMV with the symmetric-normalized adjacency
A_hat = D^-1/2 (A+I) D^-1/2:

    s   = A_hat @ x                    (scalar per node)
    z   = f(s)   where f(t) = sum_k W2[k] * relu(W1[k]*t + b1[k])
    out = A_hat @ z + b2

Sharding: nodes are range-sharded by destination across the 8 cores; all
in-edges of a node live on its owner core.  Within a core, nodes are sorted
by in-degree (descending) and assigned slots COLUMN-MAJOR across the 128
SBUF partitions (rank k -> partition k%128, column k//128), so round r (the
r-th in-edge of every node that has one) occupies only ceil(n_r/128) columns
where n_r = #nodes with degree > r.  The routed edge tables are therefore
packed (~1570 columns incl. tier padding vs ~2850 for row-major ELL), and
the self-loop contribution is just one more width-98 block in the table.

Round blocks are grouped into uniform-width tiers by a small DP.  Tier 0
(full-width blocks) is segment-summed by an in-place pairwise bf16 tree
(contiguous tensor_tensor adds, 2x DVE mode); narrow tail tiers use either a
tree or a single strided tensor_reduce over a [128, u, g] access pattern.

Per-edge normalization dinv[src] = rsqrt(1 + deg[src]) runs on the SCALAR
engine as exp(-0.5*ln(1+deg)) -- Ln and Exp share one activation-table set
(one ACT_TABLE_LOAD, prefetched by a dummy warm-up during the input DMAs),
keeping the vector engine free (Rsqrt/Reciprocal activations are blocked in
this Bass version and vector.reciprocal costs ~18us at this size).  Degrees
are routed as uint8, values as bf16; layer 1's per-edge multiply runs in
pure bf16 (2x mode).  Tier-0 is split into two chunks so scalar Ln/Exp,
the vector multiply/fold, and the (single-queue-latency-bound) input DMAs
pipeline; layer 2 splits its loads across both HWDGE queues (SP + ACT).

Hard-won correctness details: (1) consecutive dependent vector ops need
explicit same-engine semaphore self-waits (SELF_WAITS) -- without them the
emitted program computes wrong results deterministically; (2) each DMA gets
its own semaphore (completion order across one queue is not guaranteed to
match dispatch order); (3) the sync engine must WAIT for the final output
DMA's completion semaphore before the program ends, otherwise the NEFF
halts with the DMA in flight and the host result fetch reads stale bytes
(sparse, flaky corruption); (4) layer 1 ships w in bf16 (it is routed as
bf16 anyway) and dinv early on the ACT queue, hidden under the folds.

Execution is two SPMD launches (one per GCN layer) with host routing of the
gathered source features between them (np.take on the layer-1 activations),
exactly as the raw features are routed for layer 1.  All arithmetic runs on
the NeuronCores.  Measured: ~42us total (launch0 ~23.5us + launch1 ~18.5us)
vs the 86.5us baseline, rel err 2.4e-3 (gate 2e-2).
"""

import os
import numpy as np
import ml_dtypes

from concourse import bass, mybir
from concourse.bass_utils import run_bass_kernel_spmd

dt = mybir.dt
BF16 = ml_dtypes.bfloat16

NCORES = 8
N = 100000
P = 128            # SBUF partitions
CPN = 98           # node columns per partition
NPC = P * CPN      # 12544 nodes per core
SENT = NCORES * NPC  # sentinel table slot (value/deg = 0)

LAST_RESULTS = None  # list of BassKernelResults from the most recent run


def _choose_tiers(widths):
    """Group blocks (widths descending) into tiers of uniform width.
    DP minimizing ~ns: per-column cost (mult+reduce+dma) + per-tier cost."""
    B = len(widths)
    COL_NS = 2.3          # extra cost per padded column (vector+scalar+dma)
    TIER_NS = 330.0       # reduce + add instruction overhead per extra tier
    INF = float("inf")
    best = [INF] * (B + 1)
    prev = [0] * (B + 1)
    best[0] = 0.0
    for j in range(1, B + 1):
        for i in range(j):
            # tier covering blocks i..j-1, width = widths[i] (descending)
            c = best[i] + (j - i) * widths[i] * COL_NS + TIER_NS
            if c < best[j]:
                best[j] = c
                prev[j] = i
    cuts = []
    j = B
    while j > 0:
        i = prev[j]
        cuts.append((i, j))
        j = i
    cuts.reverse()
    # tier list: (col_offset, g, u); block r col start
    tiers = []
    block_col = [0] * B
    off = 0
    for (i, j) in cuts:
        u = widths[i]
        g = j - i
        for r in range(i, j):
            block_col[r] = off + (r - i) * u
        tiers.append((off, g, u))
        off += g * u
    return tiers, block_col, off


def _preprocess(x, edge_index):
    """Host routing/layout: shard by destination, degree-sort nodes
    (column-major slot order), build packed per-round source-index tables."""
    x = np.asarray(x, dtype=np.float32).reshape(-1)
    ei = np.asarray(edge_index)
    src_g = ei[0].astype(np.int64)
    dst_g = ei[1].astype(np.int64)

    cnt_g = np.bincount(dst_g, minlength=N).astype(np.int64)  # in-degree

    order_c, rank_c = [], []
    deg_sorted_c = []
    pp = np.empty(N, dtype=np.int64)  # global node -> table position
    for c in range(NCORES):
        lo, hi = c * NPC, min((c + 1) * NPC, N)
        nreal = hi - lo
        deg_local = np.zeros(NPC, dtype=np.int64)
        deg_local[:nreal] = cnt_g[lo:hi]
        order = np.argsort(-deg_local, kind="stable")
        rank = np.empty(NPC, dtype=np.int64)
        rank[order] = np.arange(NPC)
        order_c.append(order)
        rank_c.append(rank)
        deg_sorted_c.append(deg_local[order])
        pp[lo:hi] = c * NPC + rank[:nreal]

    K = int(max(int(d[0]) for d in deg_sorted_c))  # global max in-degree

    # per-round packed widths (max over cores)
    w_r = np.zeros(K, dtype=np.int64)
    for c in range(NCORES):
        ds = deg_sorted_c[c]
        for r in range(K):
            n_r = int(np.searchsorted(-ds, -(r + 1), side="right"))  # #deg>r
            w_r[r] = max(w_r[r], (n_r + P - 1) // P)

    # block 0 = self-loop block (all NPC nodes -> width CPN), then rounds
    widths = [CPN] + [int(w) for w in w_r]
    tiers, block_col, W = _choose_tiers(widths)

    # routed-table index matrices [P, W], sentinel-padded
    owner = dst_g // NPC
    idx_c = []
    for c in range(NCORES):
        lo = c * NPC
        idx_mat = np.full((P, W), SENT, dtype=np.int64)
        # self block (block 0): slot k holds node with rank k
        k_all = np.arange(NPC)
        idx_mat[k_all % P, block_col[0] + k_all // P] = c * NPC + k_all
        # edge rounds
        m = owner == c
        s_e = pp[src_g[m]]
        d_e = dst_g[m] - lo
        rj = rank_c[c][d_e]
        o = np.argsort(rj, kind="stable")
        rj_s = rj[o]
        s_s = s_e[o]
        occ = np.arange(len(rj_s)) - np.searchsorted(rj_s, rj_s)  # round idx
        cols = np.asarray(block_col, dtype=np.int64)[occ + 1] + rj_s // P
        idx_mat[rj_s % P, cols] = s_s
        idx_c.append(np.ascontiguousarray(idx_mat))

    return idx_c, order_c, cnt_g, tiers, W, K


SELF_WAITS = True  # vector self-waits: DVE drains its pipe between ops
                    # (enforced output hazard), so HW doesn't need them; set
                    # True when running under CoreSim (its race detector does
                    # not model the drain).


def _emit_tree(vector, vw, v_inc, Y, base, g, u):
    """In-place pairwise fold of g contiguous width-u blocks of Y starting at
    column `base` (pure bf16, 2x DVE mode).  Root lands at [base, base+u)."""
    n = g
    while n > 1:
        h = n // 2
        k = n - h
        vw()
        v_inc(vector.tensor_tensor(
            out=Y[:, base:base + h * u],
            in0=Y[:, base:base + h * u],
            in1=Y[:, base + k * u:base + n * u],
            op=mybir.AluOpType.add))
        n = k


def _emit_small_tiers(vector, vw, v_inc, Y, PT, tiers):
    """Fold tiers[1:] into the width-u0 root at Y[:, 0:u] (all bf16)."""
    for t in range(1, len(tiers)):
        off, g, u = tiers[t]
        if g == 1:
            vw()
            v_inc(vector.tensor_tensor(
                out=Y[:, 0:u], in0=Y[:, 0:u], in1=Y[:, off:off + u],
                op=mybir.AluOpType.add))
        elif g * u >= 128:
            _emit_tree(vector, vw, v_inc, Y, off, g, u)
            vw()
            v_inc(vector.tensor_tensor(
                out=Y[:, 0:u], in0=Y[:, 0:u], in1=Y[:, off:off + u],
                op=mybir.AluOpType.add))
        else:
            ap3 = Y[:, off:off + g * u].rearrange("p (g u) -> p u g", u=u)
            vw()
            v_inc(vector.tensor_reduce(
                out=PT[:, 0:u], in_=ap3,
                axis=mybir.AxisListType.X, op=mybir.AluOpType.add))
            vw()
            v_inc(vector.tensor_tensor(
                out=Y[:, 0:u], in0=Y[:, 0:u], in1=PT[:, 0:u],
                op=mybir.AluOpType.add))


def _tier0_groups(tiers):
    """Split tier 0's g0 blocks into pipeline groups: a small first group so
    the scalar chain starts on minimal data, then two roughly equal halves.
    Returns list of (col_lo, col_hi, nblocks)."""
    off0, g0, u0 = tiers[0]
    if g0 >= 4:
        s0 = max(2, (g0 * 3 + 2) // 5)
        sizes = [s0, g0 - s0]
    elif g0 >= 2:
        sizes = [g0 // 2, g0 - g0 // 2]
    else:
        sizes = [g0]
    groups = []
    b = 0
    for s in sizes:
        groups.append((b * u0, (b + s) * u0, s))
        b += s
    return groups


def _build_layer1(tiers, W, A, B, terms):
    """Layer 1: inputs v_ell (bf16) + c_ell (u8), both [P, W] packed tables
    including the self block.  Output [P, 2*CPN] = [w_own | dinv_own]."""
    nc = bass.Bass(num_devices=NCORES)
    nc._allow_low_precision_reason = "bf16 segment-sum within 2e-2 tolerance"

    ve_in = nc.declare_dram_parameter("v_ell", [P, W], dt.bfloat16, isOutput=False)
    ce_in = nc.declare_dram_parameter("c_ell", [P, W], dt.uint8, isOutput=False)
    out_ext = nc.declare_dram_parameter("out", [P, CPN], dt.bfloat16, isOutput=True)
    outd_ext = nc.declare_dram_parameter("outd", [P, CPN], dt.float32, isOutput=True)

    self_off = tiers[0][0]  # col offset of self block (= 0)
    off0, g0, u0 = tiers[0]
    groups = _tier0_groups(tiers)
    cC = (g0 * u0, W)
    ng = len(groups)
    umax = max([u for (_, g, u) in tiers[1:]] or [1])

    sem_cms = [nc.alloc_semaphore(f"sc{i}") for i in range(ng + 1)]
    sem_vms = [nc.alloc_semaphore(f"sx{i}") for i in range(ng + 1)]

    with (
        nc.sbuf_tensor("VE", [P, W], dt.bfloat16) as VE,
        nc.sbuf_tensor("CE", [P, W], dt.uint8) as CE,
        nc.sbuf_tensor("LN", [P, W], dt.float32) as LN,
        nc.sbuf_tensor("DE", [P, W], dt.bfloat16) as DE,
        nc.sbuf_tensor("PT", [P, umax], dt.bfloat16) as PT,
        nc.sbuf_tensor("ts", [P, CPN], dt.float32) as ts,
        nc.sbuf_tensor("to", [P, CPN], dt.float32) as to,
        nc.sbuf_tensor("tz", [P, CPN], dt.float32) as tz,
        nc.sbuf_tensor("OUT", [P, CPN], dt.bfloat16) as OUT,
        nc.sbuf_tensor("OUTD", [P, CPN], dt.float32) as OUTD,
        nc.sbuf_tensor("WRM", [P, 1], dt.float32) as WRM,
        nc.sbuf_tensor("WSCR", [P, 1], dt.float32) as WSCR,
        nc.semaphore("ss") as ss,
        nc.semaphore("sv") as sv,
        nc.semaphore("sout") as sout,
        nc.semaphore("so2") as so2,
        nc.Block() as block,
    ):
        sv_n = [0]
        ss_n = [0]
        sca = [sem_cms[i] for i in range(ng)] + [sem_cms[ng]]
        sva = [sem_vms[i] for i in range(ng)] + [sem_vms[ng]]
        ranges = [(lo, hi) for (lo, hi, _) in groups] + [cC]

        def v_inc(inst):
            inst.then_inc(sv, 1)
            sv_n[0] += 1
            return sv_n[0]

        def s_inc(inst):
            inst.then_inc(ss, 1)
            ss_n[0] += 1
            return ss_n[0]

        @block.scalar
        def _(scalar):
            # warm up the ln/exp table set while the DMAs stream (WSCR is a
            # never-written scratch; scale=0 makes the value irrelevant)
            s_inc(scalar.activation(WRM[:, 0:1], WSCR[:, :],
                                    mybir.ActivationFunctionType.Exp,
                                    bias=0.0, scale=0.0))
            marks = []
            for i, (lo, hi) in enumerate(ranges):
                scalar.wait_ge(sca[i], 16)
                cl = s_inc(scalar.activation(LN[:, lo:hi], CE[:, lo:hi],
                                             mybir.ActivationFunctionType.Ln,
                                             bias=1.0))
                scalar.wait_ge(ss, cl)
                marks.append(s_inc(scalar.activation(
                    DE[:, lo:hi], LN[:, lo:hi],
                    mybir.ActivationFunctionType.Exp, scale=-0.5)))
            # dinv_own from the self block of LN -> OUT[:, CPN:]
            dmark = s_inc(scalar.activation(
                OUTD[:, :], LN[:, self_off:self_off + CPN],
                mybir.ActivationFunctionType.Exp, scale=-0.5))
            marks.append(dmark)
            # ship dinv early on the ACT queue (overlaps the folds); wait for
            # it so the NEFF cannot halt with the DMA in flight
            scalar.wait_ge(ss, dmark)
            scalar.dma_start(out=outd_ext[:, :],
                             in_=OUTD[:, :]).then_inc(so2, 16)
            scalar.wait_ge(so2, 16)
            block.ss_marks = marks

        @block.vector
        def _(vector):
            dinv = OUTD[:, :]
            marks = block.ss_marks

            def vw():
                if SELF_WAITS and sv_n[0]:
                    vector.wait_ge(sv, sv_n[0])

            for i, (lo, hi, g) in enumerate(groups):
                vector.wait_ge(ss, marks[i])
                vector.wait_ge(sva[i], 16)
                vw()
                v_inc(vector.tensor_tensor(
                    out=VE[:, lo:hi], in0=VE[:, lo:hi],
                    in1=DE[:, lo:hi], op=mybir.AluOpType.mult))
            _emit_tree(vector, vw, v_inc, VE, 0, g0, u0)
            # chunk C: remaining tiers, accumulated into the bf16 root
            vector.wait_ge(ss, marks[ng])
            vector.wait_ge(sva[ng], 16)
            if cC[1] > cC[0]:
                vw()
                v_inc(vector.tensor_tensor(
                    out=VE[:, cC[0]:cC[1]], in0=VE[:, cC[0]:cC[1]],
                    in1=DE[:, cC[0]:cC[1]], op=mybir.AluOpType.mult))
            _emit_small_tiers(vector, vw, v_inc, VE, PT, tiers)
            # epilogue: s = dinv * fold ; z = f(s) ; w = dinv * z
            vector.wait_ge(ss, marks[ng + 1])
            vw()
            v_inc(vector.tensor_tensor(
                out=ts[:, :], in0=VE[:, 0:CPN], in1=dinv,
                op=mybir.AluOpType.mult))
            if terms is None:
                # z = (A-B)*relu(s) + B*s
                vw()
                v_inc(vector.tensor_scalar(
                    to[:, :], ts[:, :], 0.0, float(A - B),
                    mybir.AluOpType.max, mybir.AluOpType.mult))
                vw()
                v_inc(vector.scalar_tensor_tensor(
                    out=tz[:, :], in0=ts[:, :], scalar=float(B), in1=to[:, :],
                    op0=mybir.AluOpType.mult, op1=mybir.AluOpType.add))
            else:
                v_inc(vector.memset(tz[:, :], 0.0))
                for (w1k, b1k, w2k) in terms:
                    vw()
                    v_inc(vector.tensor_scalar(
                        to[:, :], ts[:, :], float(w1k), float(b1k),
                        mybir.AluOpType.mult, mybir.AluOpType.add))
                    vw()
                    v_inc(vector.tensor_scalar_max(to[:, :], to[:, :], 0.0))
                    vw()
                    v_inc(vector.scalar_tensor_tensor(
                        out=tz[:, :], in0=to[:, :], scalar=float(w2k),
                        in1=tz[:, :],
                        op0=mybir.AluOpType.mult, op1=mybir.AluOpType.add))
            vw()
            v_inc(vector.tensor_tensor(
                out=OUT[:, :], in0=tz[:, :], in1=dinv,
                op=mybir.AluOpType.mult))

        @block.sync
        def _(sync):
            # need-ordered interleave: CE_i gates the scalar chain, VE_i the
            # vector chain one act later
            for i, (lo, hi) in enumerate(ranges):
                sync.dma_start(out=CE[:, lo:hi],
                               in_=ce_in[:, lo:hi]).then_inc(sca[i], 16)
                sync.dma_start(out=VE[:, lo:hi],
                               in_=ve_in[:, lo:hi]).then_inc(sva[i], 16)
            sync.wait_ge(ss, ss_n[0])
            sync.wait_ge(sv, sv_n[0])
            sync.dma_start(out=out_ext[:, :],
                           in_=OUT[:, :]).then_inc(sout, 16)
            # the NEFF must not halt before the output DMA lands -- the host
            # result fetch races an in-flight DMA otherwise
            sync.wait_ge(sout, 16)

    return nc


def _build_layer2(tiers, W, b2val):
    """Layer 2: inputs w_ell (bf16, [P, W] packed incl. self block) and
    dinv_own (f32).  out = dinv * (segment sum) + b2.
    No scalar-engine compute; the scalar engine dispatches half the DMAs so
    both HWDGE queues stream in parallel."""
    nc = bass.Bass(num_devices=NCORES)
    nc._allow_low_precision_reason = "bf16 segment-sum within 2e-2 tolerance"

    we_in = nc.declare_dram_parameter("w_ell", [P, W], dt.bfloat16, isOutput=False)
    dd_in = nc.declare_dram_parameter("dinv", [P, CPN], dt.float32, isOutput=False)
    out_ext = nc.declare_dram_parameter("out", [P, CPN], dt.float32, isOutput=True)

    off0, g0, u0 = tiers[0]
    groups = _tier0_groups(tiers)
    cC = (g0 * u0, W)
    ng = len(groups)
    umax = max([u for (_, g, u) in tiers[1:]] or [1])

    sem_w = [nc.alloc_semaphore(f"sw{i}") for i in range(ng + 1)]

    with (
        nc.sbuf_tensor("WE", [P, W], dt.bfloat16) as WE,
        nc.sbuf_tensor("DD", [P, CPN], dt.float32) as DD,
        nc.sbuf_tensor("PT", [P, umax], dt.bfloat16) as PT,
        nc.sbuf_tensor("OUT", [P, CPN], dt.float32) as OUT,
        nc.semaphore("sdd") as sdd,
        nc.semaphore("sv") as sv,
        nc.semaphore("sout") as sout,
        nc.Block() as block,
    ):
        sv_n = [0]
        ranges = [(lo, hi) for (lo, hi, _) in groups] + [cC]

        def v_inc(inst):
            inst.then_inc(sv, 1)
            sv_n[0] += 1
            return sv_n[0]

        half = (g0 // 2) * u0

        @block.scalar
        def _(scalar):
            # second HWDGE queue: second half of tier 0 + tail tiers + dinv
            scalar.dma_start(out=WE[:, half:g0 * u0],
                             in_=we_in[:, half:g0 * u0]).then_inc(sem_w[1], 16)
            scalar.dma_start(out=WE[:, cC[0]:W],
                             in_=we_in[:, cC[0]:W]).then_inc(sem_w[2], 16)
            scalar.dma_start(out=DD[:, :], in_=dd_in[:, :]).then_inc(sdd, 16)

        @block.vector
        def _(vector):
            def vw():
                if SELF_WAITS and sv_n[0]:
                    vector.wait_ge(sv, sv_n[0])

            vector.wait_ge(sem_w[0], 16)
            vector.wait_ge(sem_w[1], 16)
            _emit_tree(vector, vw, v_inc, WE, 0, g0, u0)
            vector.wait_ge(sem_w[2], 16)
            _emit_small_tiers(vector, vw, v_inc, WE, PT, tiers)
            vector.wait_ge(sdd, 16)
            vw()
            v_inc(vector.tensor_tensor(
                out=OUT[:, :], in0=WE[:, 0:CPN], in1=DD[:, :],
                op=mybir.AluOpType.mult))
            if b2val != 0.0:
                vw()
                v_inc(vector.tensor_scalar_add(OUT[:, :], OUT[:, :],
                                               float(b2val)))

        @block.sync
        def _(sync):
            sync.dma_start(out=WE[:, 0:half],
                           in_=we_in[:, 0:half]).then_inc(sem_w[0], 16)
            sync.wait_ge(sv, sv_n[0])
            sync.dma_start(out=out_ext[:, :], in_=OUT[:, :]).then_inc(sout, 16)
            sync.wait_ge(sout, 16)

    return nc


def kernel(x, edge_index, W1, b1, W2, b2):
    global LAST_RESULTS
    idx_c, order_c, cnt_g, tiers, W, K = _preprocess(x, edge_index)

    w1 = np.asarray(W1, dtype=np.float64).reshape(-1)
    w2 = np.asarray(W2, dtype=np.float64).reshape(-1)
    b1v = np.asarray(b1, dtype=np.float64).reshape(-1)
    b2v = float(np.asarray(b2, dtype=np.float64).reshape(-1)[0])
    if np.all(b1v == 0.0):
        A = float(np.sum(w2 * w1 * (w1 > 0)))
        B = float(np.sum(w2 * w1 * (w1 < 0)))
        terms = None
    else:
        A = B = 0.0
        terms = [(float(w1[k]), float(b1v[k]), float(w2[k]))
                 for k in range(len(w1))]

    # routed tables (slot-rank order per core) + sentinel 0
    xf = np.asarray(x, dtype=np.float32).reshape(-1)
    x_tab = np.zeros(SENT + 1, dtype=np.float32)
    c_tab = np.zeros(SENT + 1, dtype=np.int64)
    for c in range(NCORES):
        lo, hi = c * NPC, min((c + 1) * NPC, N)
        xv = np.zeros(NPC, dtype=np.float32)
        xv[:hi - lo] = xf[lo:hi]
        dv = np.zeros(NPC, dtype=np.int64)
        dv[:hi - lo] = cnt_g[lo:hi]
        x_tab[c * NPC:(c + 1) * NPC] = xv[order_c[c]]
        c_tab[c * NPC:(c + 1) * NPC] = dv[order_c[c]]
    x_tab16 = x_tab.astype(BF16)
    c_tab8 = c_tab.astype(np.uint8)

    trace = bool(os.environ.get("BASS_TRACE"))

    # ---- layer 1 ----
    nc1 = _build_layer1(tiers, W, A, B, terms)
    maps1 = [{
        "v_ell": np.ascontiguousarray(x_tab16[idx_c[c]]),
        "c_ell": np.ascontiguousarray(c_tab8[idx_c[c]]),
    } for c in range(NCORES)]
    res1 = run_bass_kernel_spmd(nc1, maps1, list(range(NCORES)), trace=trace)

    # host routes layer-1 message values to edge slots (halo exchange)
    w_tab16 = np.zeros(SENT + 1, dtype=BF16)
    dd_c = []
    for c in range(NCORES):
        o = np.asarray(res1.results[c]["out"])   # bf16 w
        w_tab16[c * NPC:(c + 1) * NPC] = o.T.ravel()
        dd_c.append(np.ascontiguousarray(np.asarray(res1.results[c]["outd"])))

    # ---- layer 2 ----
    nc2 = _build_layer2(tiers, W, b2v)
    maps2 = [{
        "w_ell": np.ascontiguousarray(w_tab16[idx_c[c]]),
        "dinv": dd_c[c],
    } for c in range(NCORES)]
    res2 = run_bass_kernel_spmd(nc2, maps2, list(range(NCORES)), trace=trace)

    LAST_RESULTS = [res1, res2]

    out = np.empty((N, 1), dtype=np.float32)
    for c in range(NCORES):
        lo, hi = c * NPC, min((c + 1) * NPC, N)
        o_ranked = np.asarray(res2.results[c]["out"]).T.ravel()  # value by rank
        node_of_rank = order_c[c]          # rank -> local node id
        vals = np.empty(NPC, dtype=np.float32)
        vals[node_of_rank] = o_ranked      # local node id -> value
        out[lo:hi, 0] = vals[:hi - lo]
    return out
